# revision 43
# baseline (speedup 1.0000x reference)
"""Causal self-attention with RoPE (B=2, T=2048, C=2048, 16 heads) on 8 TRN2
NeuronCores.

Sharding: data-parallel over batch x tensor-parallel over heads.
Core c handles batch c//4 and heads 4*(c%4) .. 4*(c%4)+4. Each core computes
its heads' q/k/v projections, RoPE, causal attention, and a partial output
projection over its heads' channels; the host sums the 4 partial projections
per batch (the tensor-parallel reduce) and stacks the batches.

Per-core program (fp16 operands, fp32 accumulation):
  A1: qT/kT[hd, T] = (w_qk chunk).T @ xT accumulated over C chunks in PSUM.
      The first 8 tiles run contraction-outer so compute starts on chunk 0
      and hides the xT DMA; the rest run contraction-inner. RoPE is applied
      via one ACT fp16 copy out of PSUM, two fp16 DVE products against
      stacked [cos;sin]/[sin;cos] tables, and two DVE half-combines. q/k
      weight columns are host-permuted even-first so rotation pairs sit in
      partition halves.
  B:  per (head, 512-wide q tile): for each causal 128-chunk of k:
      scoresT = kT_chunk.T @ qT_tile -> PSUM (diagonal chunks only over the
      causal q-range); att = exp(scale*scoresT) (ACT); the 128-wide diagonal
      block is triangle-masked (DVE); att accumulated into att_acc (DVE
      fp16); yT += v_chunk.T @ att in PSUM with score matmuls emitted two
      chunks ahead so PE never waits on exp. The softmax denominator is a
      GPSIMD partition_all_reduce of att_acc; yT *= 1/denom (DVE).
      The v projection v[T, hd*4] = (xT chunk).T @ w_v and the output
      projection outT[C, T] partial = (w_proj chunk).T @ yT are pumped into
      PE slots the exp-bound attention pipeline leaves idle: v tiles are
      dribbled between chunks (first two ride A1's PSUM ring), and the
      previous q tile's projection fills each head boundary.
"""
import os
import numpy as np
from contextlib import ExitStack

os.environ.setdefault("JAX_COMPILATION_CACHE_DIR", "/tmp/jax_comp_cache")

import concourse.bass as bass
import concourse.tile as tile
from concourse import bacc, mybir
from concourse import bass_isa
from concourse.bass_utils import run_bass_kernel_spmd
from concourse._compat import axon_active

FP16 = mybir.dt.float16
FP32 = mybir.dt.float32

B, T, C, NH = 2, 2048, 2048, 16
HD = C // NH
N_CORES = 8
GROUPS = N_CORES // B
HPC = NH // GROUPS
QN = 512
AN = 1024


def _build_nc(T=2048, C=2048, HPC=4, n_cores=8, qn=512, reps=1, an=AN):
    """HPC = heads per core; head_dim fixed 128. qn = moving free-dim tile."""
    HD = 128
    CH = C // 128          # contraction chunks
    QK_COLS = HPC * HD     # q cols (= k cols) per core
    V_COLS = HPC * HD
    NQT = T // qn          # q tiles in attention
    NKC = T // 128         # k chunks
    scale = 1.0 / np.sqrt(np.float32(HD))

    nc = bacc.Bacc("TRN2", target_bir_lowering=False, debug=False,
                   num_devices=n_cores)
    xT_ap = nc.dram_tensor("xT", (C, T), FP16, kind="ExternalInput").ap()
    wqk_ap = nc.dram_tensor("wqk", (C, 2 * QK_COLS), FP16, kind="ExternalInput").ap()
    wv_ap = nc.dram_tensor("wv", (C, V_COLS), FP16, kind="ExternalInput").ap()
    wp_ap = nc.dram_tensor("wp", (HPC * HD, C), FP16, kind="ExternalInput").ap()
    cs2_ap = nc.dram_tensor("cs2", (128, T), FP16, kind="ExternalInput").ap()
    sc2_ap = nc.dram_tensor("sc2", (128, T), FP16, kind="ExternalInput").ap()
    mask_ap = nc.dram_tensor("masks", (128, 4 * qn), FP16, kind="ExternalInput").ap()
    out_ap = nc.dram_tensor("outT", (C, T), FP16, kind="ExternalOutput").ap()


    with tile.TileContext(nc) as tc:
      for rep in range(reps):
        R = f"r{rep}_"
        with ExitStack() as top:
            xt_pool = top.enter_context(tc.tile_pool(name=R+"xt", bufs=1))
            qk_pool = top.enter_context(tc.tile_pool(name=R+"qk", bufs=1))

            xt = xt_pool.tile([128, CH, T], FP16)
            qk_sb = qk_pool.tile([128, 2 * HPC, T], FP16)   # [hd, col, T]; cols 0..HPC-1 q, HPC.. k
            v_pool = top.enter_context(tc.tile_pool(name=R+"v", bufs=1))
            v_sb = v_pool.tile([128, NKC, V_COLS], FP16)   # [t_lo, t_chunk, vcol]
            wv_pool = top.enter_context(tc.tile_pool(name=R+"wv", bufs=1))
            wv = wv_pool.tile([128, CH, V_COLS], FP16)

            # ---- Phase A1: q/k projection + RoPE ----
            with ExitStack() as sA1:
                wqk_pool = sA1.enter_context(tc.tile_pool(name=R+"wqk", bufs=1))
                tab_pool = sA1.enter_context(tc.tile_pool(name=R+"tab", bufs=1))
                qraw_pool = sA1.enter_context(tc.tile_pool(name=R+"qraw", bufs=3))
                tmp_pool = sA1.enter_context(tc.tile_pool(name=R+"tmp", bufs=3))
                psA_pool = sA1.enter_context(tc.tile_pool(name=R+"psA", bufs=8, space="PSUM"))

                wqk = wqk_pool.tile([128, CH, 2 * QK_COLS], FP16)
                cs2_t = tab_pool.tile([128, T], FP16)   # [cos_h; sin_h]
                sc2_t = tab_pool.tile([128, T], FP16)   # [sin_h; cos_h]
                warm = tab_pool.tile([1, 1], FP32)
                nc.vector.memset(warm[:], 0.0)
                warm2 = tab_pool.tile([1, 1], FP32)
                nc.scalar.activation(warm2[:], warm[:],
                                     mybir.ActivationFunctionType.Exp)
                GC = 4 * HD
                wqk_r = wqk_ap.rearrange("(ch p) n -> p ch n", p=128)
                xt_r = xT_ap.rearrange("(ch p) t -> p ch t", p=128)
                for ch in range(CH):
                    # 256-column pieces keep DMA descriptors >= 512B
                    nc.sync.dma_start(wqk[:, ch, 0:GC], wqk_r[:, ch, 0:GC])
                    if ch == 0:
                        for tq in range(T // qn):
                            nc.sync.dma_start(xt[:, ch, tq * qn:(tq + 1) * qn],
                                              xt_r[:, ch, tq * qn:(tq + 1) * qn])
                    else:
                        nc.sync.dma_start(xt[:, ch, 0:T // 2], xt_r[:, ch, 0:T // 2])
                        nc.sync.dma_start(xt[:, ch, T // 2:T], xt_r[:, ch, T // 2:T])
                nc.sync.dma_start(cs2_t[:], cs2_ap[:])
                nc.sync.dma_start(sc2_t[:], sc2_ap[:])
                # remaining q/k weight columns
                for ch in range(CH):
                    nc.sync.dma_start(wqk[:, ch, GC:2 * QK_COLS],
                                      wqk_r[:, ch, GC:2 * QK_COLS])
                for ch in range(CH):
                    nc.sync.dma_start(wv[:, ch, :],
                                      wv_ap.rearrange("(ch p) n -> p ch n", p=128)[:, ch, :])

                def rope_emit(col, tq, ps):
                    # two SBUF inputs of a TensorTensor must share their base
                    # partition, so each half-product reads table and operand
                    # at the same offset and lands in a base-0 temp; outputs
                    # may sit at a different partition offset.
                    sl = slice(tq * qn, (tq + 1) * qn)
                    dst = qk_sb[:, col, sl]
                    qraw = qraw_pool.tile([128, qn], FP16, tag="qraw",
                                          name=f"{R}qraw_{col}_{tq}")
                    nc.scalar.copy(qraw[:], ps[:])
                    tm = [tmp_pool.tile([64, qn], FP16, tag=f"t{i}",
                                        name=f"{R}t{i}_{col}_{tq}")
                          for i in range(4)]
                    nc.vector.tensor_mul(tm[0][:], qraw[0:64, :], cs2_t[0:64, sl])
                    nc.vector.tensor_mul(tm[1][:], qraw[64:128, :], cs2_t[64:128, sl])
                    nc.vector.tensor_mul(tm[2][:], qraw[0:64, :], sc2_t[0:64, sl])
                    nc.vector.tensor_mul(tm[3][:], qraw[64:128, :], sc2_t[64:128, sl])
                    nc.vector.tensor_sub(dst[0:64, :], tm[0][:], tm[1][:])
                    nc.vector.tensor_add(dst[64:128, :], tm[2][:], tm[3][:])

                # group: cols 0-3 x first two q tiles (contraction-outer,
                # hides the xT DMA); then cols 0-3's other q tiles (their
                # weights are already resident), then cols 4-7
                grp = [(col, tq) for col in range(4) for tq in range(2)]
                tiles = grp \
                    + [(col, tq) for col in range(4) for tq in range(2, T // qn)] \
                    + [(col, tq) for col in range(4, 2 * HPC) for tq in range(T // qn)]
                grp_ps = []
                for (col, tq) in grp:
                    gps = psA_pool.tile([128, qn], FP32, tag="psA",
                                        name=f"{R}psA_{col}_{tq}")
                    grp_ps.append(gps)
                for ch in range(CH):
                    for gi, (col, tq) in enumerate(grp):
                        nc.tensor.matmul(
                            grp_ps[gi][:], wqk[:, ch, col * HD:(col + 1) * HD],
                            xt[:, ch, tq * qn:(tq + 1) * qn],
                            start=(ch == 0), stop=(ch == CH - 1))
                for gi, (col, tq) in enumerate(grp):
                    rope_emit(col, tq, grp_ps[gi])
                for (col, tq) in tiles[len(grp):]:
                    ps = psA_pool.tile([128, qn], FP32, tag="psA",
                                       name=f"{R}psA_{col}_{tq}")
                    for ch in range(CH):
                        nc.tensor.matmul(
                            ps[:], wqk[:, ch, col * HD:(col + 1) * HD],
                            xt[:, ch, tq * qn:(tq + 1) * qn],
                            start=(ch == 0), stop=(ch == CH - 1))
                    rope_emit(col, tq, ps)

                # first two v tiles ride the psA ring so PE stays busy while
                # phase B's PSUM pools wait for A1's last tiles to drain
                for ts in range(2):
                    psv = psA_pool.tile([128, V_COLS], FP32, tag="psA",
                                        name=f"{R}psVa_{ts}")
                    for ch in range(CH):
                        nc.tensor.matmul(
                            psv[:], xt[:, ch, ts * 128:(ts + 1) * 128],
                            wv[:, ch, :],
                            start=(ch == 0), stop=(ch == CH - 1))
                    nc.scalar.copy(v_sb[:, ts, :], psv[:])

            # ---- Phase B: attention, with the v projection and the C
            # projection pumped into PE slots left idle by the exp-bound
            # attention pipeline ----
            if True:
                with ExitStack() as sY:
                    yt_pool = sY.enter_context(tc.tile_pool(name=R+"yt", bufs=1))
                    yt_sb = yt_pool.tile([128, HPC, T], FP16)
                    with ExitStack() as sB:
                        msk_pool = sB.enter_context(tc.tile_pool(name=R+"msk", bufs=1))
                        att_pool = sB.enter_context(tc.tile_pool(name=R+"att", bufs=5))
                        acc_pool = sB.enter_context(tc.tile_pool(name=R+"acc", bufs=2))
                        rec_pool = sB.enter_context(tc.tile_pool(name=R+"rec", bufs=2))
                        psS_pool = sB.enter_context(tc.tile_pool(name=R+"psS", bufs=3, space="PSUM"))
                        psY_pool = sB.enter_context(tc.tile_pool(name=R+"psY", bufs=2, space="PSUM"))

                        wp_pool = sB.enter_context(tc.tile_pool(name=R+"wp", bufs=1))
                        o_pool = sB.enter_context(tc.tile_pool(name=R+"o", bufs=4))
                        psO_pool = sB.enter_context(tc.tile_pool(name=R+"psO", bufs=3, space="PSUM"))
                        msk = msk_pool.tile([128, 4, qn], FP16)
                        nc.sync.dma_start(msk[:], mask_ap.rearrange("p (j n) -> p j n", n=qn))
                        wp = wp_pool.tile([128, HPC, C], FP16)
                        nc.sync.dma_start(wp[:], wp_ap.rearrange("(hh p) c -> p hh c", p=128))

                        diag_per_qt = qn // 128
                        NCT = C // 128

                        # v-projection pump: one 128-row v tile accumulates in
                        # a psO-ring bank; its matmuls are dribbled into PE
                        # slots between attention chunks.
                        v_queue = []
                        v_active = [None, 0]

                        def v_pump(budget):
                            while budget > 0:
                                if v_active[0] is None:
                                    if not v_queue:
                                        return
                                    ts = v_queue.pop(0)
                                    psv = psO_pool.tile([128, V_COLS], FP32,
                                                        tag="psO",
                                                        name=f"{R}psV_{ts}")
                                    v_active[0] = (ts, psv)
                                    v_active[1] = 0
                                ts, psv = v_active[0]
                                ch = v_active[1]
                                nc.tensor.matmul(
                                    psv[:], xt[:, ch, ts * 128:(ts + 1) * 128],
                                    wv[:, ch, :],
                                    start=(ch == 0), stop=(ch == CH - 1))
                                v_active[1] += 1
                                budget -= 1
                                if v_active[1] == CH:
                                    nc.scalar.copy(v_sb[:, ts, :], psv[:])
                                    v_active[0] = None

                        def proj_emit(qt, cts):
                            """C-projection matmuls for q tile qt over column
                            chunks cts; PSUM->SBUF copies alternate DVE/Pool."""
                            for ct in cts:
                                pso = psO_pool.tile([128, 512], FP32, tag="psO",
                                                    name=f"{R}psO_{ct}_{qt}")
                                for hh in range(HPC):
                                    nc.tensor.matmul(
                                        pso[:], wp[:, hh, ct * 128:(ct + 1) * 128],
                                        yt_sb[:, hh, qt * qn:(qt + 1) * qn],
                                        start=(hh == 0), stop=(hh == HPC - 1))
                                o_t = o_pool.tile([128, 512], FP16, tag="o",
                                                  name=f"{R}o_{ct}_{qt}")
                                if ct % 2 == 0:
                                    nc.scalar.copy(o_t[:], pso[:])
                                else:
                                    nc.vector.tensor_copy(o_t[:], pso[:])
                                nc.sync.dma_start(
                                    out_ap[ct * 128:(ct + 1) * 128, qt * qn:(qt + 1) * qn],
                                    o_t[:])

                        # v tiles 2..3 must precede attention (0-1 were done
                        # at the A1 tail); the rest are pumped during earlier
                        # q tiles' chunk loops.
                        v_queue.extend(range(2, NKC))
                        v_pump(2 * CH)
                        V_RATE = {0: 4, 1: 2, 2: 2, 3: 0}

                        SKEW = 3
                        for qt in range(NQT):
                            for h in range(HPC):
                                nch = diag_per_qt * (qt + 1)
                                psY = psY_pool.tile([128, qn], FP32, tag="psY",
                                                    name=f"{R}psY_{h}_{qt}")
                                acc = acc_pool.tile([128, qn], FP16, tag="acc",
                                                    name=f"{R}acc_{h}_{qt}")
                                atts = [None] * nch
                                offs = [None] * nch

                                def score_emit(kc):
                                    # diagonal chunks only compute the causal
                                    # q-range [qoff:]; the 128-wide diagonal
                                    # block is masked with a triangle
                                    j = kc - diag_per_qt * qt
                                    qoff = 128 * j if j > 0 else 0
                                    sl = slice(qoff, qn)
                                    psS = psS_pool.tile([128, qn], FP32, tag="psS",
                                                        name=f"{R}psS_{h}_{qt}_{kc}")
                                    nc.tensor.matmul(
                                        psS[:, sl],
                                        qk_sb[:, HPC + h, kc * 128:(kc + 1) * 128],
                                        qk_sb[:, h, qt * qn + qoff:(qt + 1) * qn],
                                        start=True, stop=True)
                                    att = att_pool.tile([128, qn], FP16, tag="att",
                                                        name=f"{R}att_{h}_{qt}_{kc}")
                                    nc.scalar.activation(
                                        att[:, sl], psS[:, sl],
                                        mybir.ActivationFunctionType.Exp,
                                        scale=float(scale))
                                    if j >= 0:
                                        nc.vector.tensor_mul(
                                            att[:, qoff:qoff + 128],
                                            att[:, qoff:qoff + 128],
                                            msk[:, 0, 0:128])
                                    if kc == 0:
                                        nc.vector.tensor_copy(acc[:], att[:])
                                    else:
                                        nc.vector.tensor_add(acc[:, sl], acc[:, sl],
                                                             att[:, sl])
                                    atts[kc] = att
                                    offs[kc] = sl

                                def y_emit(kc):
                                    sl = offs[kc]
                                    nc.tensor.matmul(
                                        psY[:, sl], v_sb[:, kc, h * HD:(h + 1) * HD],
                                        atts[kc][:, sl],
                                        start=(kc == 0), stop=(kc == nch - 1))

                                for kc in range(nch):
                                    score_emit(kc)
                                    if kc >= SKEW:
                                        y_emit(kc - SKEW)
                                    v_pump(V_RATE[qt])
                                for kc in range(max(0, nch - SKEW), nch):
                                    y_emit(kc)

                                # previous q tile's projection fills PE while
                                # the last chunk's exp/acc chain drains
                                if qt > 0:
                                    proj_emit(qt - 1, range(h * (NCT // HPC),
                                                            (h + 1) * (NCT // HPC)))
                                else:
                                    v_pump(CH // 2)
                                # softmax denominator: cross-partition sum of
                                # acc on the Pool engine (keeps PE free)
                                dsum = rec_pool.tile([128, qn], FP32, tag="dsum",
                                                     name=f"{R}dsum_{h}_{qt}")
                                nc.gpsimd.partition_all_reduce(
                                    dsum[:], acc[:], channels=128,
                                    reduce_op=bass_isa.ReduceOp.add)
                                recb = rec_pool.tile([128, qn], FP32, tag="recb",
                                                     name=f"{R}recb_{h}_{qt}")
                                nc.vector.reciprocal(recb[:], dsum[:])
                                dst = yt_sb[:, h, qt * qn:(qt + 1) * qn]
                                nc.vector.tensor_mul(dst, psY[:], recb[:])
                            if qt == NQT - 1:
                                proj_emit(qt, range(NCT))
    nc.compile()
    return nc


_CACHE = {}


def _rope_tables_np(t_len, hd):
    inv_freq = 1.0 / (10000.0 ** (np.arange(0, hd, 2, dtype=np.float32) / hd))
    t = np.arange(t_len, dtype=np.float32)
    freqs = np.outer(t, inv_freq)
    emb = np.concatenate([freqs, freqs], axis=-1)
    return np.cos(emb)[:, ::2].astype(np.float32), np.sin(emb)[:, ::2].astype(np.float32)


def _static_arrays():
    if "static" not in _CACHE:
        cos_, sin_ = _rope_tables_np(T, HD)   # (T, hd/2) each
        cosT = np.ascontiguousarray(cos_.T)   # (64, T)
        sinT = np.ascontiguousarray(sin_.T)
        cs2 = np.concatenate([cosT, sinT], axis=0).astype(np.float16)  # (128, T)
        sc2 = np.concatenate([sinT, cosT], axis=0).astype(np.float16)
        perm = np.concatenate([np.arange(0, HD, 2), np.arange(1, HD, 2)])
        p = np.arange(128)[:, None]
        f = np.arange(QN)[None, :]
        masks = np.concatenate(
            [(p <= (f - 128 * j)).astype(np.float16) for j in range(QN // 128)],
            axis=1)
        _CACHE["static"] = (cs2, sc2, perm, masks)
    return _CACHE["static"]


def _host_prep(x, w_qkv, w_proj):
    cs2, sc2, perm, masks = _static_arrays()

    wq = w_qkv[:, 0 * C:1 * C]
    wk = w_qkv[:, 1 * C:2 * C]
    wv = w_qkv[:, 2 * C:3 * C]

    in_maps = []
    for c in range(N_CORES):
        b = c // GROUPS
        hg = c % GROUPS
        hs = slice(hg * HPC * HD, (hg + 1) * HPC * HD)
        wq_c = wq[:, hs].reshape(C, HPC, HD)[:, :, perm].reshape(C, HPC * HD)
        wk_c = wk[:, hs].reshape(C, HPC, HD)[:, :, perm].reshape(C, HPC * HD)
        in_maps.append({
            "xT": np.ascontiguousarray(x[b].T).astype(np.float16),
            "wqk": np.concatenate([wq_c, wk_c], axis=1).astype(np.float16),
            "wv": np.ascontiguousarray(wv[:, hs]).astype(np.float16),
            "wp": np.ascontiguousarray(w_proj[hs, :]).astype(np.float16),
            "cs2": cs2,
            "sc2": sc2,
            "masks": masks,
        })
    return in_maps


class _PjrtRunner:
    """Caches the jitted shard_map callable so repeat kernel() calls skip
    retracing. Mirrors concourse.bass2jax.run_bass_via_pjrt."""

    def __init__(self, nc):
        import jax
        from jax.sharding import Mesh, PartitionSpec, NamedSharding
        from jax.experimental.shard_map import shard_map
        from concourse.bass2jax import (
            _bass_exec_p, install_neuronx_cc_hook, partition_id_tensor)

        install_neuronx_cc_hook()
        self.jax = jax
        partition_name = nc.partition_id_tensor.name if nc.partition_id_tensor else None
        in_names, out_names, out_avals = [], [], []
        for alloc in nc.m.functions[0].allocations:
            if not isinstance(alloc, mybir.MemoryLocationSet):
                continue
            name = alloc.memorylocations[0].name
            if alloc.kind == "ExternalInput":
                if name != partition_name:
                    in_names.append(name)
            elif alloc.kind == "ExternalOutput":
                out_names.append(name)
                out_avals.append(jax.core.ShapedArray(
                    tuple(alloc.tensor_shape), mybir.dt.np(alloc.dtype)))
        self.in_names, self.out_names, self.out_avals = in_names, out_names, out_avals
        n_params = len(in_names)
        n_outs = len(out_avals)
        bind_names = tuple(in_names + out_names +
                           ([partition_name] if partition_name else []))
        donate = tuple(range(n_params, n_params + n_outs))

        def _body(*args):
            operands = list(args)
            if partition_name:
                operands.append(partition_id_tensor())
            outs = _bass_exec_p.bind(
                *operands,
                out_avals=tuple(out_avals),
                in_names=bind_names,
                out_names=tuple(out_names),
                lowering_input_output_aliases=(),
                sim_require_finite=True,
                sim_require_nnan=True,
                nc=nc,
            )
            return tuple(outs)

        devices = jax.devices()[:N_CORES]
        self.mesh = Mesh(np.asarray(devices), ("core",))
        self.sharding = NamedSharding(self.mesh, PartitionSpec("core"))
        in_specs = (PartitionSpec("core"),) * (n_params + n_outs)
        out_specs = (PartitionSpec("core"),) * len(out_names)
        self.fn = jax.jit(
            shard_map(_body, mesh=self.mesh, in_specs=in_specs,
                      out_specs=out_specs, check_rep=False),
            donate_argnums=donate,
        )

    def run(self, in_maps):
        jax = self.jax
        concat = [
            np.concatenate([np.asarray(m[name]) for m in in_maps], axis=0)
            for name in self.in_names
        ]
        dev = [jax.device_put(c, self.sharding) for c in concat]
        zeros = [
            jax.device_put(
                np.zeros((N_CORES * a.shape[0], *a.shape[1:]), a.dtype),
                self.sharding)
            for a in self.out_avals
        ]
        outs = self.fn(*dev, *zeros)
        jax.block_until_ready(outs)
        res = []
        for c in range(N_CORES):
            d = {}
            for i, name in enumerate(self.out_names):
                a = np.asarray(outs[i])
                d[name] = a.reshape(N_CORES, *self.out_avals[i].shape)[c]
            res.append(d)
        return res


def _get_rt():
    if "rt" not in _CACHE:
        nc = _build_nc(T=T, C=C, HPC=HPC, n_cores=N_CORES, qn=QN, reps=1, an=AN)
        _CACHE["nc"] = nc
        _CACHE["rt"] = _PjrtRunner(nc) if axon_active() else None
    return _CACHE.get("nc"), _CACHE.get("rt")


def kernel(x, w_qkv, w_proj, n_head):
    assert int(n_head) == NH
    x = np.asarray(x, dtype=np.float32)
    w_qkv = np.asarray(w_qkv, dtype=np.float32)
    w_proj = np.asarray(w_proj, dtype=np.float32)
    assert x.shape == (B, T, C) and w_qkv.shape == (C, 3 * C) and w_proj.shape == (C, C)

    nc, rt = _get_rt()
    in_maps = _host_prep(x, w_qkv, w_proj)
    if rt is not None:
        results = rt.run(in_maps)
    else:
        results = run_bass_kernel_spmd(nc, in_maps,
                                       core_ids=list(range(N_CORES))).results

    out = np.zeros((B, T, C), dtype=np.float32)
    for c in range(N_CORES):
        b = c // GROUPS
        out[b] += results[c]["outT"].astype(np.float32).T
    return out


# revision 46
# speedup vs baseline: 1.1852x; 1.1852x over previous
"""Causal self-attention with RoPE (B=2, T=2048, C=2048, 16 heads) on 8 TRN2
NeuronCores.

Sharding: data-parallel over batch x tensor-parallel over heads.
Core c handles batch c//4 and heads 4*(c%4) .. 4*(c%4)+4. Each core computes
its heads' q/k/v projections, RoPE, causal attention, and a partial output
projection over its heads' channels; the host sums the 4 partial projections
per batch (the tensor-parallel reduce) and stacks the batches.

Per-core program (fp16 operands, fp32 accumulation):
  A1: qT/kT[hd, T] = (w_qk chunk).T @ xT accumulated over C chunks in PSUM.
      The first 8 tiles run contraction-outer so compute starts on chunk 0
      and hides the xT DMA; the rest run contraction-inner. RoPE is applied
      via one ACT fp16 copy out of PSUM, two fp16 DVE products against
      stacked [cos;sin]/[sin;cos] tables, and two DVE half-combines. q/k
      weight columns are host-permuted even-first so rotation pairs sit in
      partition halves.
  B:  per (head, 512-wide q tile): for each causal 128-chunk of k:
      scoresT = kT_chunk.T @ qT_tile -> PSUM (diagonal chunks only over the
      causal q-range); att = exp(scale*scoresT) (ACT); the 128-wide diagonal
      block is triangle-masked (DVE); att accumulated into att_acc (DVE
      fp16); yT += v_chunk.T @ att in PSUM with score matmuls emitted two
      chunks ahead so PE never waits on exp. The softmax denominator is a
      GPSIMD partition_all_reduce of att_acc; yT *= 1/denom (DVE).
      The v projection v[T, hd*4] = (xT chunk).T @ w_v and the output
      projection outT[C, T] partial = (w_proj chunk).T @ yT are pumped into
      PE slots the exp-bound attention pipeline leaves idle: v tiles are
      dribbled between chunks (first two ride A1's PSUM ring), and the
      previous q tile's projection fills each head boundary.
"""
import os
import numpy as np
from contextlib import ExitStack

os.environ.setdefault("JAX_COMPILATION_CACHE_DIR", "/tmp/jax_comp_cache")

import concourse.bass as bass
import concourse.tile as tile
from concourse import bacc, mybir
from concourse import bass_isa
from concourse.bass_utils import run_bass_kernel_spmd
from concourse._compat import axon_active

FP16 = mybir.dt.float16
FP32 = mybir.dt.float32

B, T, C, NH = 2, 2048, 2048, 16
HD = C // NH
N_CORES = 8
GROUPS = N_CORES // B
HPC = NH // GROUPS
QN = 512
AN = 1024


def _build_nc(T=2048, C=2048, HPC=4, n_cores=8, qn=512, reps=1, an=AN):
    """HPC = heads per core; head_dim fixed 128. qn = moving free-dim tile."""
    HD = 128
    CH = C // 128          # contraction chunks
    QK_COLS = HPC * HD     # q cols (= k cols) per core
    V_COLS = HPC * HD
    NQT = T // qn          # q tiles in attention
    NKC = T // 128         # k chunks
    scale = 1.0 / np.sqrt(np.float32(HD))

    nc = bacc.Bacc("TRN2", target_bir_lowering=False, debug=False,
                   num_devices=n_cores)
    xT_ap = nc.dram_tensor("xT", (C, T), FP16, kind="ExternalInput").ap()
    wqk_ap = nc.dram_tensor("wqk", (C, 2 * QK_COLS), FP16, kind="ExternalInput").ap()
    wv_ap = nc.dram_tensor("wv", (C, V_COLS), FP16, kind="ExternalInput").ap()
    wp_ap = nc.dram_tensor("wp", (HPC * HD, C), FP16, kind="ExternalInput").ap()
    cs2_ap = nc.dram_tensor("cs2", (128, T), FP16, kind="ExternalInput").ap()
    sc2_ap = nc.dram_tensor("sc2", (128, T), FP16, kind="ExternalInput").ap()
    mask_ap = nc.dram_tensor("masks", (128, 4 * qn), FP16, kind="ExternalInput").ap()
    out_ap = nc.dram_tensor("outT", (C, T), FP16, kind="ExternalOutput").ap()


    with tile.TileContext(nc) as tc:
      for rep in range(reps):
        R = f"r{rep}_"
        with ExitStack() as top:
            xt_pool = top.enter_context(tc.tile_pool(name=R+"xt", bufs=1))
            qk_pool = top.enter_context(tc.tile_pool(name=R+"qk", bufs=1))

            xt = xt_pool.tile([128, CH, T], FP16)
            qk_sb = qk_pool.tile([128, 2 * HPC, T], FP16)   # [hd, col, T]; cols 0..HPC-1 q, HPC.. k
            v_pool = top.enter_context(tc.tile_pool(name=R+"v", bufs=1))
            v_sb = v_pool.tile([128, NKC, V_COLS], FP16)   # [t_lo, t_chunk, vcol]
            wv_pool = top.enter_context(tc.tile_pool(name=R+"wv", bufs=1))
            wv = wv_pool.tile([128, CH, V_COLS], FP16)

            # ---- Phase A1: q/k projection + RoPE ----
            with ExitStack() as sA1:
                wqk_pool = sA1.enter_context(tc.tile_pool(name=R+"wqk", bufs=1))
                tab_pool = sA1.enter_context(tc.tile_pool(name=R+"tab", bufs=1))
                qraw_pool = sA1.enter_context(tc.tile_pool(name=R+"qraw", bufs=3))
                tmp_pool = sA1.enter_context(tc.tile_pool(name=R+"tmp", bufs=3))
                psA_pool = sA1.enter_context(tc.tile_pool(name=R+"psA", bufs=8, space="PSUM"))

                wqk = wqk_pool.tile([128, CH, 2 * QK_COLS], FP16)
                cs2_t = tab_pool.tile([128, T], FP16)   # [cos_h; sin_h]
                sc2_t = tab_pool.tile([128, T], FP16)   # [sin_h; cos_h]
                warm = tab_pool.tile([1, 1], FP32)
                nc.vector.memset(warm[:], 0.0)
                warm2 = tab_pool.tile([1, 1], FP32)
                nc.scalar.activation(warm2[:], warm[:],
                                     mybir.ActivationFunctionType.Exp)
                GC = 4 * HD
                wqk_r = wqk_ap.rearrange("(ch p) n -> p ch n", p=128)
                xt_r = xT_ap.rearrange("(ch p) t -> p ch t", p=128)
                for ch in range(CH):
                    # 256-column pieces keep DMA descriptors >= 512B
                    nc.sync.dma_start(wqk[:, ch, 0:GC], wqk_r[:, ch, 0:GC])
                    if ch == 0:
                        for tq in range(T // qn):
                            nc.sync.dma_start(xt[:, ch, tq * qn:(tq + 1) * qn],
                                              xt_r[:, ch, tq * qn:(tq + 1) * qn])
                    else:
                        nc.sync.dma_start(xt[:, ch, 0:T // 2], xt_r[:, ch, 0:T // 2])
                        nc.sync.dma_start(xt[:, ch, T // 2:T], xt_r[:, ch, T // 2:T])
                nc.sync.dma_start(cs2_t[:], cs2_ap[:])
                nc.sync.dma_start(sc2_t[:], sc2_ap[:])
                # remaining q/k weight columns
                for ch in range(CH):
                    nc.sync.dma_start(wqk[:, ch, GC:2 * QK_COLS],
                                      wqk_r[:, ch, GC:2 * QK_COLS])
                for ch in range(CH):
                    nc.sync.dma_start(wv[:, ch, :],
                                      wv_ap.rearrange("(ch p) n -> p ch n", p=128)[:, ch, :])

                def rope_emit(col, tq, ps):
                    # two SBUF inputs of a TensorTensor must share their base
                    # partition, so each half-product reads table and operand
                    # at the same offset and lands in a base-0 temp; outputs
                    # may sit at a different partition offset.
                    sl = slice(tq * qn, (tq + 1) * qn)
                    dst = qk_sb[:, col, sl]
                    qraw = qraw_pool.tile([128, qn], FP16, tag="qraw",
                                          name=f"{R}qraw_{col}_{tq}")
                    nc.scalar.copy(qraw[:], ps[:])
                    tm = [tmp_pool.tile([64, qn], FP16, tag=f"t{i}",
                                        name=f"{R}t{i}_{col}_{tq}")
                          for i in range(4)]
                    nc.vector.tensor_mul(tm[0][:], qraw[0:64, :], cs2_t[0:64, sl])
                    nc.vector.tensor_mul(tm[1][:], qraw[64:128, :], cs2_t[64:128, sl])
                    nc.vector.tensor_mul(tm[2][:], qraw[0:64, :], sc2_t[0:64, sl])
                    nc.vector.tensor_mul(tm[3][:], qraw[64:128, :], sc2_t[64:128, sl])
                    nc.vector.tensor_sub(dst[0:64, :], tm[0][:], tm[1][:])
                    nc.vector.tensor_add(dst[64:128, :], tm[2][:], tm[3][:])

                # group: cols 0-3 x first two q tiles (contraction-outer,
                # hides the xT DMA); then cols 0-3's other q tiles (their
                # weights are already resident), then cols 4-7
                grp = [(col, tq) for col in range(4) for tq in range(2)]
                tiles = grp \
                    + [(col, tq) for col in range(4) for tq in range(2, T // qn)] \
                    + [(col, tq) for col in range(4, 2 * HPC) for tq in range(T // qn)]
                grp_ps = []
                for (col, tq) in grp:
                    gps = psA_pool.tile([128, qn], FP32, tag="psA",
                                        name=f"{R}psA_{col}_{tq}")
                    grp_ps.append(gps)
                for ch in range(CH):
                    for gi, (col, tq) in enumerate(grp):
                        nc.tensor.matmul(
                            grp_ps[gi][:], wqk[:, ch, col * HD:(col + 1) * HD],
                            xt[:, ch, tq * qn:(tq + 1) * qn],
                            start=(ch == 0), stop=(ch == CH - 1))
                for gi, (col, tq) in enumerate(grp):
                    rope_emit(col, tq, grp_ps[gi])
                for (col, tq) in tiles[len(grp):]:
                    ps = psA_pool.tile([128, qn], FP32, tag="psA",
                                       name=f"{R}psA_{col}_{tq}")
                    for ch in range(CH):
                        nc.tensor.matmul(
                            ps[:], wqk[:, ch, col * HD:(col + 1) * HD],
                            xt[:, ch, tq * qn:(tq + 1) * qn],
                            start=(ch == 0), stop=(ch == CH - 1))
                    rope_emit(col, tq, ps)

                # first two v tiles ride the psA ring so PE stays busy while
                # phase B's PSUM pools wait for A1's last tiles to drain
                for ts in range(2):
                    psv = psA_pool.tile([128, V_COLS], FP32, tag="psA",
                                        name=f"{R}psVa_{ts}")
                    for ch in range(CH):
                        nc.tensor.matmul(
                            psv[:], xt[:, ch, ts * 128:(ts + 1) * 128],
                            wv[:, ch, :],
                            start=(ch == 0), stop=(ch == CH - 1))
                    nc.scalar.copy(v_sb[:, ts, :], psv[:])

            # ---- Phase B: attention, with the v projection and the C
            # projection pumped into PE slots left idle by the exp-bound
            # attention pipeline ----
            if True:
                with ExitStack() as sY:
                    yt_pool = sY.enter_context(tc.tile_pool(name=R+"yt", bufs=1))
                    yt_sb = yt_pool.tile([128, HPC, T], FP16)
                    with ExitStack() as sB:
                        msk_pool = sB.enter_context(tc.tile_pool(name=R+"msk", bufs=1))
                        att_pool = sB.enter_context(tc.tile_pool(name=R+"att", bufs=5))
                        acc_pool = sB.enter_context(tc.tile_pool(name=R+"acc", bufs=2))
                        rec_pool = sB.enter_context(tc.tile_pool(name=R+"rec", bufs=2))
                        psS_pool = sB.enter_context(tc.tile_pool(name=R+"psS", bufs=3, space="PSUM"))
                        psY_pool = sB.enter_context(tc.tile_pool(name=R+"psY", bufs=2, space="PSUM"))

                        wp_pool = sB.enter_context(tc.tile_pool(name=R+"wp", bufs=1))
                        o_pool = sB.enter_context(tc.tile_pool(name=R+"o", bufs=4))
                        psO_pool = sB.enter_context(tc.tile_pool(name=R+"psO", bufs=3, space="PSUM"))
                        msk = msk_pool.tile([128, 4, qn], FP16)
                        nc.sync.dma_start(msk[:], mask_ap.rearrange("p (j n) -> p j n", n=qn))
                        wp = wp_pool.tile([128, HPC, C], FP16)
                        nc.sync.dma_start(wp[:], wp_ap.rearrange("(hh p) c -> p hh c", p=128))

                        diag_per_qt = qn // 128
                        NCT = C // 128

                        # v-projection pump: one 128-row v tile accumulates in
                        # a psO-ring bank; its matmuls are dribbled into PE
                        # slots between attention chunks.
                        v_queue = []
                        v_active = [None, 0]

                        def v_pump(budget):
                            while budget > 0:
                                if v_active[0] is None:
                                    if not v_queue:
                                        return
                                    ts = v_queue.pop(0)
                                    psv = psO_pool.tile([128, V_COLS], FP32,
                                                        tag="psO",
                                                        name=f"{R}psV_{ts}")
                                    v_active[0] = (ts, psv)
                                    v_active[1] = 0
                                ts, psv = v_active[0]
                                ch = v_active[1]
                                nc.tensor.matmul(
                                    psv[:], xt[:, ch, ts * 128:(ts + 1) * 128],
                                    wv[:, ch, :],
                                    start=(ch == 0), stop=(ch == CH - 1))
                                v_active[1] += 1
                                budget -= 1
                                if v_active[1] == CH:
                                    nc.scalar.copy(v_sb[:, ts, :], psv[:])
                                    v_active[0] = None

                        def proj_emit(qt, cts):
                            """C-projection matmuls for q tile qt over column
                            chunks cts; PSUM->SBUF copies alternate DVE/Pool."""
                            for ct in cts:
                                pso = psO_pool.tile([128, 512], FP32, tag="psO",
                                                    name=f"{R}psO_{ct}_{qt}")
                                for hh in range(HPC):
                                    nc.tensor.matmul(
                                        pso[:], wp[:, hh, ct * 128:(ct + 1) * 128],
                                        yt_sb[:, hh, qt * qn:(qt + 1) * qn],
                                        start=(hh == 0), stop=(hh == HPC - 1))
                                o_t = o_pool.tile([128, 512], FP16, tag="o",
                                                  name=f"{R}o_{ct}_{qt}")
                                if ct % 2 == 0:
                                    nc.scalar.copy(o_t[:], pso[:])
                                else:
                                    nc.vector.tensor_copy(o_t[:], pso[:])
                                nc.sync.dma_start(
                                    out_ap[ct * 128:(ct + 1) * 128, qt * qn:(qt + 1) * qn],
                                    o_t[:])

                        # v tiles 2..3 must precede attention (0-1 were done
                        # at the A1 tail); the rest are pumped during earlier
                        # q tiles' chunk loops.
                        v_queue.extend(range(2, NKC))
                        v_pump(2 * CH)
                        V_RATE = {0: 4, 1: 2, 2: 2, 3: 0}

                        SKEW = 3
                        for qt in range(NQT):
                            for h in range(HPC):
                                nch = diag_per_qt * (qt + 1)
                                psY = psY_pool.tile([128, qn], FP32, tag="psY",
                                                    name=f"{R}psY_{h}_{qt}")
                                acc = acc_pool.tile([128, qn], FP16, tag="acc",
                                                    name=f"{R}acc_{h}_{qt}")
                                atts = [None] * nch
                                offs = [None] * nch

                                def score_emit(kc):
                                    # diagonal chunks only compute the causal
                                    # q-range [qoff:]; the 128-wide diagonal
                                    # block is masked with a triangle
                                    j = kc - diag_per_qt * qt
                                    qoff = 128 * j if j > 0 else 0
                                    sl = slice(qoff, qn)
                                    psS = psS_pool.tile([128, qn], FP32, tag="psS",
                                                        name=f"{R}psS_{h}_{qt}_{kc}")
                                    nc.tensor.matmul(
                                        psS[:, sl],
                                        qk_sb[:, HPC + h, kc * 128:(kc + 1) * 128],
                                        qk_sb[:, h, qt * qn + qoff:(qt + 1) * qn],
                                        start=True, stop=True)
                                    att = att_pool.tile([128, qn], FP16, tag="att",
                                                        name=f"{R}att_{h}_{qt}_{kc}")
                                    nc.scalar.activation(
                                        att[:, sl], psS[:, sl],
                                        mybir.ActivationFunctionType.Exp,
                                        scale=float(scale))
                                    if j >= 0:
                                        nc.vector.tensor_mul(
                                            att[:, qoff:qoff + 128],
                                            att[:, qoff:qoff + 128],
                                            msk[:, 0, 0:128])
                                    if kc == 0:
                                        nc.vector.tensor_copy(acc[:], att[:])
                                    else:
                                        nc.vector.tensor_add(acc[:, sl], acc[:, sl],
                                                             att[:, sl])
                                    atts[kc] = att
                                    offs[kc] = sl

                                def y_emit(kc):
                                    sl = offs[kc]
                                    nc.tensor.matmul(
                                        psY[:, sl], v_sb[:, kc, h * HD:(h + 1) * HD],
                                        atts[kc][:, sl],
                                        start=(kc == 0), stop=(kc == nch - 1))

                                cts_fill = (list(range(h * (NCT // HPC),
                                                       (h + 1) * (NCT // HPC)))
                                            if qt > 0 else [])
                                for kc in range(nch):
                                    score_emit(kc)
                                    if kc >= SKEW:
                                        y_emit(kc - SKEW)
                                    v_pump(V_RATE[qt])
                                    if kc >= nch - 2 and cts_fill:
                                        proj_emit(qt - 1, [cts_fill.pop(0)])
                                for kc in range(max(0, nch - SKEW), nch):
                                    y_emit(kc)
                                if cts_fill:
                                    proj_emit(qt - 1, cts_fill)
                                if qt == 0:
                                    v_pump(CH // 2)
                                # softmax denominator: cross-partition sum of
                                # acc on the Pool engine (keeps PE free)
                                dsum = rec_pool.tile([128, qn], FP32, tag="dsum",
                                                     name=f"{R}dsum_{h}_{qt}")
                                nc.gpsimd.partition_all_reduce(
                                    dsum[:], acc[:], channels=128,
                                    reduce_op=bass_isa.ReduceOp.add)
                                recb = rec_pool.tile([128, qn], FP32, tag="recb",
                                                     name=f"{R}recb_{h}_{qt}")
                                nc.vector.reciprocal(recb[:], dsum[:])
                                dst = yt_sb[:, h, qt * qn:(qt + 1) * qn]
                                nc.vector.tensor_mul(dst, psY[:], recb[:])
                            if qt == NQT - 1:
                                proj_emit(qt, range(NCT))
    nc.compile()
    return nc


_CACHE = {}


def _rope_tables_np(t_len, hd):
    inv_freq = 1.0 / (10000.0 ** (np.arange(0, hd, 2, dtype=np.float32) / hd))
    t = np.arange(t_len, dtype=np.float32)
    freqs = np.outer(t, inv_freq)
    emb = np.concatenate([freqs, freqs], axis=-1)
    return np.cos(emb)[:, ::2].astype(np.float32), np.sin(emb)[:, ::2].astype(np.float32)


def _static_arrays():
    if "static" not in _CACHE:
        cos_, sin_ = _rope_tables_np(T, HD)   # (T, hd/2) each
        cosT = np.ascontiguousarray(cos_.T)   # (64, T)
        sinT = np.ascontiguousarray(sin_.T)
        cs2 = np.concatenate([cosT, sinT], axis=0).astype(np.float16)  # (128, T)
        sc2 = np.concatenate([sinT, cosT], axis=0).astype(np.float16)
        perm = np.concatenate([np.arange(0, HD, 2), np.arange(1, HD, 2)])
        p = np.arange(128)[:, None]
        f = np.arange(QN)[None, :]
        masks = np.concatenate(
            [(p <= (f - 128 * j)).astype(np.float16) for j in range(QN // 128)],
            axis=1)
        _CACHE["static"] = (cs2, sc2, perm, masks)
    return _CACHE["static"]


def _host_prep(x, w_qkv, w_proj):
    cs2, sc2, perm, masks = _static_arrays()

    wq = w_qkv[:, 0 * C:1 * C]
    wk = w_qkv[:, 1 * C:2 * C]
    wv = w_qkv[:, 2 * C:3 * C]

    in_maps = []
    for c in range(N_CORES):
        b = c // GROUPS
        hg = c % GROUPS
        hs = slice(hg * HPC * HD, (hg + 1) * HPC * HD)
        wq_c = wq[:, hs].reshape(C, HPC, HD)[:, :, perm].reshape(C, HPC * HD)
        wk_c = wk[:, hs].reshape(C, HPC, HD)[:, :, perm].reshape(C, HPC * HD)
        in_maps.append({
            "xT": np.ascontiguousarray(x[b].T).astype(np.float16),
            "wqk": np.concatenate([wq_c, wk_c], axis=1).astype(np.float16),
            "wv": np.ascontiguousarray(wv[:, hs]).astype(np.float16),
            "wp": np.ascontiguousarray(w_proj[hs, :]).astype(np.float16),
            "cs2": cs2,
            "sc2": sc2,
            "masks": masks,
        })
    return in_maps


class _PjrtRunner:
    """Caches the jitted shard_map callable so repeat kernel() calls skip
    retracing. Mirrors concourse.bass2jax.run_bass_via_pjrt."""

    def __init__(self, nc):
        import jax
        from jax.sharding import Mesh, PartitionSpec, NamedSharding
        from jax.experimental.shard_map import shard_map
        from concourse.bass2jax import (
            _bass_exec_p, install_neuronx_cc_hook, partition_id_tensor)

        install_neuronx_cc_hook()
        self.jax = jax
        partition_name = nc.partition_id_tensor.name if nc.partition_id_tensor else None
        in_names, out_names, out_avals = [], [], []
        for alloc in nc.m.functions[0].allocations:
            if not isinstance(alloc, mybir.MemoryLocationSet):
                continue
            name = alloc.memorylocations[0].name
            if alloc.kind == "ExternalInput":
                if name != partition_name:
                    in_names.append(name)
            elif alloc.kind == "ExternalOutput":
                out_names.append(name)
                out_avals.append(jax.core.ShapedArray(
                    tuple(alloc.tensor_shape), mybir.dt.np(alloc.dtype)))
        self.in_names, self.out_names, self.out_avals = in_names, out_names, out_avals
        n_params = len(in_names)
        n_outs = len(out_avals)
        bind_names = tuple(in_names + out_names +
                           ([partition_name] if partition_name else []))
        donate = tuple(range(n_params, n_params + n_outs))

        def _body(*args):
            operands = list(args)
            if partition_name:
                operands.append(partition_id_tensor())
            outs = _bass_exec_p.bind(
                *operands,
                out_avals=tuple(out_avals),
                in_names=bind_names,
                out_names=tuple(out_names),
                lowering_input_output_aliases=(),
                sim_require_finite=True,
                sim_require_nnan=True,
                nc=nc,
            )
            return tuple(outs)

        devices = jax.devices()[:N_CORES]
        self.mesh = Mesh(np.asarray(devices), ("core",))
        self.sharding = NamedSharding(self.mesh, PartitionSpec("core"))
        in_specs = (PartitionSpec("core"),) * (n_params + n_outs)
        out_specs = (PartitionSpec("core"),) * len(out_names)
        self.fn = jax.jit(
            shard_map(_body, mesh=self.mesh, in_specs=in_specs,
                      out_specs=out_specs, check_rep=False),
            donate_argnums=donate,
        )

    def run(self, in_maps):
        jax = self.jax
        concat = [
            np.concatenate([np.asarray(m[name]) for m in in_maps], axis=0)
            for name in self.in_names
        ]
        dev = [jax.device_put(c, self.sharding) for c in concat]
        zeros = [
            jax.device_put(
                np.zeros((N_CORES * a.shape[0], *a.shape[1:]), a.dtype),
                self.sharding)
            for a in self.out_avals
        ]
        outs = self.fn(*dev, *zeros)
        jax.block_until_ready(outs)
        res = []
        for c in range(N_CORES):
            d = {}
            for i, name in enumerate(self.out_names):
                a = np.asarray(outs[i])
                d[name] = a.reshape(N_CORES, *self.out_avals[i].shape)[c]
            res.append(d)
        return res


def _get_rt():
    if "rt" not in _CACHE:
        nc = _build_nc(T=T, C=C, HPC=HPC, n_cores=N_CORES, qn=QN, reps=1, an=AN)
        _CACHE["nc"] = nc
        _CACHE["rt"] = _PjrtRunner(nc) if axon_active() else None
    return _CACHE.get("nc"), _CACHE.get("rt")


def kernel(x, w_qkv, w_proj, n_head):
    assert int(n_head) == NH
    x = np.asarray(x, dtype=np.float32)
    w_qkv = np.asarray(w_qkv, dtype=np.float32)
    w_proj = np.asarray(w_proj, dtype=np.float32)
    assert x.shape == (B, T, C) and w_qkv.shape == (C, 3 * C) and w_proj.shape == (C, C)

    nc, rt = _get_rt()
    in_maps = _host_prep(x, w_qkv, w_proj)
    if rt is not None:
        results = rt.run(in_maps)
    else:
        results = run_bass_kernel_spmd(nc, in_maps,
                                       core_ids=list(range(N_CORES))).results

    out = np.zeros((B, T, C), dtype=np.float32)
    for c in range(N_CORES):
        b = c // GROUPS
        out[b] += results[c]["outT"].astype(np.float32).T
    return out


# revision 48
# speedup vs baseline: 1.4252x; 1.2026x over previous
"""Causal self-attention with RoPE (B=2, T=2048, C=2048, 16 heads) on 8 TRN2
NeuronCores.

Sharding: data-parallel over batch x tensor-parallel over heads.
Core c handles batch c//4 and heads 4*(c%4) .. 4*(c%4)+4. Each core computes
its heads' q/k/v projections, RoPE, causal attention, and a partial output
projection over its heads' channels; the host sums the 4 partial projections
per batch (the tensor-parallel reduce) and stacks the batches.

Per-core program (fp16 operands, fp32 accumulation):
  A1: qT/kT[hd, T] = (w_qk chunk).T @ xT accumulated over C chunks in PSUM.
      The first 8 tiles run contraction-outer so compute starts on chunk 0
      and hides the xT DMA; the rest run contraction-inner. RoPE is applied
      via one ACT fp16 copy out of PSUM, two fp16 DVE products against
      stacked [cos;sin]/[sin;cos] tables, and two DVE half-combines. q/k
      weight columns are host-permuted even-first so rotation pairs sit in
      partition halves.
  B:  per (head, 512-wide q tile): for each causal 128-chunk of k:
      scoresT = kT_chunk.T @ qT_tile -> PSUM (diagonal chunks only over the
      causal q-range); att = exp(scale*scoresT) (ACT); the 128-wide diagonal
      block is triangle-masked (DVE); att accumulated into att_acc (DVE
      fp16); yT += v_chunk.T @ att in PSUM with score matmuls emitted three
      chunks ahead so PE never waits on exp. The softmax denominator is a
      GPSIMD partition_all_reduce of att_acc; yT *= 1/denom (DVE).
      The v projection v[T, hd*4] = (xT chunk).T @ w_v and the output
      projection outT[C, T] partial = (w_proj chunk).T @ yT are pumped into
      PE slots the exp-bound attention pipeline leaves idle: v tiles are
      dribbled between chunks (first two ride A1's PSUM ring), and the
      previous q tile's projection fills each head boundary.
"""
import os
import numpy as np
from contextlib import ExitStack

os.environ.setdefault("JAX_COMPILATION_CACHE_DIR", "/tmp/jax_comp_cache")

import concourse.bass as bass
import concourse.tile as tile
from concourse import bacc, mybir
from concourse import bass_isa
from concourse.bass_utils import run_bass_kernel_spmd
from concourse._compat import axon_active

FP16 = mybir.dt.float16
FP32 = mybir.dt.float32

B, T, C, NH = 2, 2048, 2048, 16
HD = C // NH
N_CORES = 8
GROUPS = N_CORES // B
HPC = NH // GROUPS
QN = 512
AN = 1024


def _build_nc(T=2048, C=2048, HPC=4, n_cores=8, qn=512, reps=1, an=AN):
    """HPC = heads per core; head_dim fixed 128. qn = moving free-dim tile."""
    HD = 128
    CH = C // 128          # contraction chunks
    QK_COLS = HPC * HD     # q cols (= k cols) per core
    V_COLS = HPC * HD
    NQT = T // qn          # q tiles in attention
    NKC = T // 128         # k chunks
    scale = 1.0 / np.sqrt(np.float32(HD))

    nc = bacc.Bacc("TRN2", target_bir_lowering=False, debug=False,
                   num_devices=n_cores)
    xT_ap = nc.dram_tensor("xT", (C, T), FP16, kind="ExternalInput").ap()
    wqk_ap = nc.dram_tensor("wqk", (C, 2 * QK_COLS), FP16, kind="ExternalInput").ap()
    wv_ap = nc.dram_tensor("wv", (C, V_COLS), FP16, kind="ExternalInput").ap()
    wp_ap = nc.dram_tensor("wp", (HPC * HD, C), FP16, kind="ExternalInput").ap()
    cs2_ap = nc.dram_tensor("cs2", (128, T), FP16, kind="ExternalInput").ap()
    sc2_ap = nc.dram_tensor("sc2", (128, T), FP16, kind="ExternalInput").ap()
    mask_ap = nc.dram_tensor("masks", (128, 4 * qn), FP16, kind="ExternalInput").ap()
    out_ap = nc.dram_tensor("outT", (C, T), FP16, kind="ExternalOutput").ap()


    with tile.TileContext(nc) as tc:
      for rep in range(reps):
        R = f"r{rep}_"
        with ExitStack() as top:
            xt_pool = top.enter_context(tc.tile_pool(name=R+"xt", bufs=1))
            qk_pool = top.enter_context(tc.tile_pool(name=R+"qk", bufs=1))

            xt = xt_pool.tile([128, CH, T], FP16)
            qk_sb = qk_pool.tile([128, 2 * HPC, T], FP16)   # [hd, col, T]; cols 0..HPC-1 q, HPC.. k
            v_pool = top.enter_context(tc.tile_pool(name=R+"v", bufs=1))
            v_sb = v_pool.tile([128, NKC, V_COLS], FP16)   # [t_lo, t_chunk, vcol]
            wv_pool = top.enter_context(tc.tile_pool(name=R+"wv", bufs=1))
            wv = wv_pool.tile([128, CH, V_COLS], FP16)

            # ---- Phase A1: q/k projection + RoPE ----
            with ExitStack() as sA1:
                wqk_pool = sA1.enter_context(tc.tile_pool(name=R+"wqk", bufs=1))
                tab_pool = sA1.enter_context(tc.tile_pool(name=R+"tab", bufs=1))
                qraw_pool = sA1.enter_context(tc.tile_pool(name=R+"qraw", bufs=3))
                tmp_pool = sA1.enter_context(tc.tile_pool(name=R+"tmp", bufs=3))
                psA_pool = sA1.enter_context(tc.tile_pool(name=R+"psA", bufs=8, space="PSUM"))

                wqk = wqk_pool.tile([128, CH, 2 * QK_COLS], FP16)
                cs2_t = tab_pool.tile([128, T], FP16)   # [cos_h; sin_h]
                sc2_t = tab_pool.tile([128, T], FP16)   # [sin_h; cos_h]
                warm = tab_pool.tile([1, 1], FP32)
                nc.vector.memset(warm[:], 0.0)
                warm2 = tab_pool.tile([1, 1], FP32)
                nc.scalar.activation(warm2[:], warm[:],
                                     mybir.ActivationFunctionType.Exp)
                GC = 4 * HD
                wqk_r = wqk_ap.rearrange("(ch p) n -> p ch n", p=128)
                xt_r = xT_ap.rearrange("(ch p) t -> p ch t", p=128)
                for ch in range(CH):
                    # 256-column pieces keep DMA descriptors >= 512B
                    nc.sync.dma_start(wqk[:, ch, 0:GC], wqk_r[:, ch, 0:GC])
                    if ch == 0:
                        for tq in range(T // qn):
                            nc.sync.dma_start(xt[:, ch, tq * qn:(tq + 1) * qn],
                                              xt_r[:, ch, tq * qn:(tq + 1) * qn])
                    else:
                        nc.sync.dma_start(xt[:, ch, 0:T // 2], xt_r[:, ch, 0:T // 2])
                        nc.sync.dma_start(xt[:, ch, T // 2:T], xt_r[:, ch, T // 2:T])
                nc.sync.dma_start(cs2_t[:], cs2_ap[:])
                nc.sync.dma_start(sc2_t[:], sc2_ap[:])
                # remaining q/k weight columns
                for ch in range(CH):
                    nc.sync.dma_start(wqk[:, ch, GC:2 * QK_COLS],
                                      wqk_r[:, ch, GC:2 * QK_COLS])
                for ch in range(CH):
                    nc.sync.dma_start(wv[:, ch, :],
                                      wv_ap.rearrange("(ch p) n -> p ch n", p=128)[:, ch, :])

                def rope_emit(col, tq, ps):
                    # two SBUF inputs of a TensorTensor must share their base
                    # partition, so each half-product reads table and operand
                    # at the same offset and lands in a base-0 temp; outputs
                    # may sit at a different partition offset.
                    sl = slice(tq * qn, (tq + 1) * qn)
                    dst = qk_sb[:, col, sl]
                    qraw = qraw_pool.tile([128, qn], FP16, tag="qraw",
                                          name=f"{R}qraw_{col}_{tq}")
                    nc.scalar.copy(qraw[:], ps[:])
                    tm = [tmp_pool.tile([64, qn], FP16, tag=f"t{i}",
                                        name=f"{R}t{i}_{col}_{tq}")
                          for i in range(4)]
                    nc.vector.tensor_mul(tm[0][:], qraw[0:64, :], cs2_t[0:64, sl])
                    nc.vector.tensor_mul(tm[1][:], qraw[64:128, :], cs2_t[64:128, sl])
                    nc.vector.tensor_mul(tm[2][:], qraw[0:64, :], sc2_t[0:64, sl])
                    nc.vector.tensor_mul(tm[3][:], qraw[64:128, :], sc2_t[64:128, sl])
                    nc.vector.tensor_sub(dst[0:64, :], tm[0][:], tm[1][:])
                    nc.vector.tensor_add(dst[64:128, :], tm[2][:], tm[3][:])

                # group: cols 0-3 x first two q tiles (contraction-outer,
                # hides the xT DMA); then cols 0-3's other q tiles (their
                # weights are already resident), then cols 4-7
                grp = [(col, tq) for col in range(4) for tq in range(2)]
                tiles = grp \
                    + [(col, tq) for col in range(4) for tq in range(2, T // qn)] \
                    + [(col, tq) for col in range(4, 2 * HPC) for tq in range(T // qn)]
                grp_ps = []
                for (col, tq) in grp:
                    gps = psA_pool.tile([128, qn], FP32, tag="psA",
                                        name=f"{R}psA_{col}_{tq}")
                    grp_ps.append(gps)
                for ch in range(CH):
                    for gi, (col, tq) in enumerate(grp):
                        nc.tensor.matmul(
                            grp_ps[gi][:], wqk[:, ch, col * HD:(col + 1) * HD],
                            xt[:, ch, tq * qn:(tq + 1) * qn],
                            start=(ch == 0), stop=(ch == CH - 1))
                for gi, (col, tq) in enumerate(grp):
                    rope_emit(col, tq, grp_ps[gi])
                for (col, tq) in tiles[len(grp):]:
                    ps = psA_pool.tile([128, qn], FP32, tag="psA",
                                       name=f"{R}psA_{col}_{tq}")
                    for ch in range(CH):
                        nc.tensor.matmul(
                            ps[:], wqk[:, ch, col * HD:(col + 1) * HD],
                            xt[:, ch, tq * qn:(tq + 1) * qn],
                            start=(ch == 0), stop=(ch == CH - 1))
                    rope_emit(col, tq, ps)

                # first two v tiles ride the psA ring so PE stays busy while
                # phase B's PSUM pools wait for A1's last tiles to drain
                for ts in range(2):
                    psv = psA_pool.tile([128, V_COLS], FP32, tag="psA",
                                        name=f"{R}psVa_{ts}")
                    for ch in range(CH):
                        nc.tensor.matmul(
                            psv[:], xt[:, ch, ts * 128:(ts + 1) * 128],
                            wv[:, ch, :],
                            start=(ch == 0), stop=(ch == CH - 1))
                    nc.scalar.copy(v_sb[:, ts, :], psv[:])

            # ---- Phase B: attention, with the v projection and the C
            # projection pumped into PE slots left idle by the exp-bound
            # attention pipeline ----
            if True:
                with ExitStack() as sY:
                    yt_pool = sY.enter_context(tc.tile_pool(name=R+"yt", bufs=1))
                    yt_sb = yt_pool.tile([128, HPC, T], FP16)
                    with ExitStack() as sB:
                        msk_pool = sB.enter_context(tc.tile_pool(name=R+"msk", bufs=1))
                        att_pool = sB.enter_context(tc.tile_pool(name=R+"att", bufs=5))
                        acc_pool = sB.enter_context(tc.tile_pool(name=R+"acc", bufs=2))
                        rec_pool = sB.enter_context(tc.tile_pool(name=R+"rec", bufs=2))
                        psS_pool = sB.enter_context(tc.tile_pool(name=R+"psS", bufs=3, space="PSUM"))
                        psY_pool = sB.enter_context(tc.tile_pool(name=R+"psY", bufs=2, space="PSUM"))

                        wp_pool = sB.enter_context(tc.tile_pool(name=R+"wp", bufs=1))
                        o_pool = sB.enter_context(tc.tile_pool(name=R+"o", bufs=4))
                        psO_pool = sB.enter_context(tc.tile_pool(name=R+"psO", bufs=3, space="PSUM"))
                        msk = msk_pool.tile([128, 4, qn], FP16)
                        nc.sync.dma_start(msk[:], mask_ap.rearrange("p (j n) -> p j n", n=qn))
                        wp = wp_pool.tile([128, HPC, C], FP16)
                        nc.sync.dma_start(wp[:], wp_ap.rearrange("(hh p) c -> p hh c", p=128))

                        diag_per_qt = qn // 128
                        NCT = C // 128

                        # v-projection pump: one 128-row v tile accumulates in
                        # a psO-ring bank; its matmuls are dribbled into PE
                        # slots between attention chunks.
                        v_queue = []
                        v_active = [None, 0]

                        def v_pump(budget):
                            while budget > 0:
                                if v_active[0] is None:
                                    if not v_queue:
                                        return
                                    ts = v_queue.pop(0)
                                    psv = psO_pool.tile([128, V_COLS], FP32,
                                                        tag="psO",
                                                        name=f"{R}psV_{ts}")
                                    v_active[0] = (ts, psv)
                                    v_active[1] = 0
                                ts, psv = v_active[0]
                                ch = v_active[1]
                                nc.tensor.matmul(
                                    psv[:], xt[:, ch, ts * 128:(ts + 1) * 128],
                                    wv[:, ch, :],
                                    start=(ch == 0), stop=(ch == CH - 1))
                                v_active[1] += 1
                                budget -= 1
                                if v_active[1] == CH:
                                    nc.scalar.copy(v_sb[:, ts, :], psv[:])
                                    v_active[0] = None

                        out_r = out_ap.rearrange("(g p) t -> p g t", p=128)

                        def proj_emit(qt, cts, paired=False):
                            """C-projection matmuls for q tile qt over column
                            chunks cts; PSUM->SBUF copies alternate ACT/DVE.
                            paired=True (final dense block only) merges each
                            ct-pair into one output DMA to halve the HWDGE
                            serialization in the kernel tail."""
                            o2 = None
                            for ct in cts:
                                pso = psO_pool.tile([128, 512], FP32, tag="psO",
                                                    name=f"{R}psO_{ct}_{qt}")
                                for hh in range(HPC):
                                    nc.tensor.matmul(
                                        pso[:], wp[:, hh, ct * 128:(ct + 1) * 128],
                                        yt_sb[:, hh, qt * qn:(qt + 1) * qn],
                                        start=(hh == 0), stop=(hh == HPC - 1))
                                if paired:
                                    if ct % 2 == 0:
                                        o2 = o_pool.tile([128, 2, 512], FP16,
                                                         tag="o2",
                                                         name=f"{R}o2_{ct}_{qt}")
                                        nc.scalar.copy(o2[:, 0, :], pso[:])
                                    else:
                                        nc.vector.tensor_copy(o2[:, 1, :], pso[:])
                                        nc.sync.dma_start(
                                            out_r[:, ct - 1:ct + 1,
                                                  qt * qn:(qt + 1) * qn],
                                            o2[:])
                                    continue
                                o_t = o_pool.tile([128, 512], FP16, tag="o",
                                                  name=f"{R}o_{ct}_{qt}")
                                if ct % 2 == 0:
                                    nc.scalar.copy(o_t[:], pso[:])
                                else:
                                    nc.vector.tensor_copy(o_t[:], pso[:])
                                nc.sync.dma_start(
                                    out_ap[ct * 128:(ct + 1) * 128, qt * qn:(qt + 1) * qn],
                                    o_t[:])

                        # v tiles 2..3 must precede attention (0-1 were done
                        # at the A1 tail); the rest are pumped during earlier
                        # q tiles' chunk loops.
                        v_queue.extend(range(2, NKC))
                        v_pump(2 * CH)
                        V_RATE = {0: 4, 1: 2, 2: 2, 3: 0}

                        SKEW = 3
                        for qt in range(NQT):
                            for h in range(HPC):
                                nch = diag_per_qt * (qt + 1)
                                psY = psY_pool.tile([128, qn], FP32, tag="psY",
                                                    name=f"{R}psY_{h}_{qt}")
                                acc = acc_pool.tile([128, qn], FP16, tag="acc",
                                                    name=f"{R}acc_{h}_{qt}")
                                atts = [None] * nch
                                offs = [None] * nch

                                def score_emit(kc):
                                    # diagonal chunks only compute the causal
                                    # q-range [qoff:]; the 128-wide diagonal
                                    # block is masked with a triangle
                                    j = kc - diag_per_qt * qt
                                    qoff = 128 * j if j > 0 else 0
                                    sl = slice(qoff, qn)
                                    psS = psS_pool.tile([128, qn], FP32, tag="psS",
                                                        name=f"{R}psS_{h}_{qt}_{kc}")
                                    nc.tensor.matmul(
                                        psS[:, sl],
                                        qk_sb[:, HPC + h, kc * 128:(kc + 1) * 128],
                                        qk_sb[:, h, qt * qn + qoff:(qt + 1) * qn],
                                        start=True, stop=True)
                                    att = att_pool.tile([128, qn], FP16, tag="att",
                                                        name=f"{R}att_{h}_{qt}_{kc}")
                                    nc.scalar.activation(
                                        att[:, sl], psS[:, sl],
                                        mybir.ActivationFunctionType.Exp,
                                        scale=float(scale))
                                    if j >= 0:
                                        nc.vector.tensor_mul(
                                            att[:, qoff:qoff + 128],
                                            att[:, qoff:qoff + 128],
                                            msk[:, 0, 0:128])
                                    if kc == 0:
                                        nc.vector.tensor_copy(acc[:], att[:])
                                    else:
                                        nc.vector.tensor_add(acc[:, sl], acc[:, sl],
                                                             att[:, sl])
                                    atts[kc] = att
                                    offs[kc] = sl

                                def y_emit(kc):
                                    sl = offs[kc]
                                    nc.tensor.matmul(
                                        psY[:, sl], v_sb[:, kc, h * HD:(h + 1) * HD],
                                        atts[kc][:, sl],
                                        start=(kc == 0), stop=(kc == nch - 1))

                                cts_fill = (list(range(h * (NCT // HPC),
                                                       (h + 1) * (NCT // HPC)))
                                            if qt > 0 else [])
                                for kc in range(nch):
                                    score_emit(kc)
                                    if kc >= SKEW:
                                        y_emit(kc - SKEW)
                                    v_pump(V_RATE[qt])
                                    if kc >= nch - 2 and cts_fill:
                                        proj_emit(qt - 1, [cts_fill.pop(0)])
                                for kc in range(max(0, nch - SKEW), nch):
                                    y_emit(kc)
                                if cts_fill:
                                    proj_emit(qt - 1, cts_fill)
                                if qt == 0:
                                    v_pump(CH // 2)
                                # softmax denominator: cross-partition sum of
                                # acc on the Pool engine (keeps PE free)
                                dsum = rec_pool.tile([128, qn], FP32, tag="dsum",
                                                     name=f"{R}dsum_{h}_{qt}")
                                nc.gpsimd.partition_all_reduce(
                                    dsum[:], acc[:], channels=128,
                                    reduce_op=bass_isa.ReduceOp.add)
                                recb = rec_pool.tile([128, qn], FP32, tag="recb",
                                                     name=f"{R}recb_{h}_{qt}")
                                nc.vector.reciprocal(recb[:], dsum[:])
                                dst = yt_sb[:, h, qt * qn:(qt + 1) * qn]
                                nc.vector.tensor_mul(dst, psY[:], recb[:])
                            if qt == NQT - 1:
                                proj_emit(qt, range(NCT), paired=True)
    nc.compile()
    return nc


_CACHE = {}


def _rope_tables_np(t_len, hd):
    inv_freq = 1.0 / (10000.0 ** (np.arange(0, hd, 2, dtype=np.float32) / hd))
    t = np.arange(t_len, dtype=np.float32)
    freqs = np.outer(t, inv_freq)
    emb = np.concatenate([freqs, freqs], axis=-1)
    return np.cos(emb)[:, ::2].astype(np.float32), np.sin(emb)[:, ::2].astype(np.float32)


def _static_arrays():
    if "static" not in _CACHE:
        cos_, sin_ = _rope_tables_np(T, HD)   # (T, hd/2) each
        cosT = np.ascontiguousarray(cos_.T)   # (64, T)
        sinT = np.ascontiguousarray(sin_.T)
        cs2 = np.concatenate([cosT, sinT], axis=0).astype(np.float16)  # (128, T)
        sc2 = np.concatenate([sinT, cosT], axis=0).astype(np.float16)
        perm = np.concatenate([np.arange(0, HD, 2), np.arange(1, HD, 2)])
        p = np.arange(128)[:, None]
        f = np.arange(QN)[None, :]
        masks = np.concatenate(
            [(p <= (f - 128 * j)).astype(np.float16) for j in range(QN // 128)],
            axis=1)
        _CACHE["static"] = (cs2, sc2, perm, masks)
    return _CACHE["static"]


def _host_prep(x, w_qkv, w_proj):
    cs2, sc2, perm, masks = _static_arrays()

    wq = w_qkv[:, 0 * C:1 * C]
    wk = w_qkv[:, 1 * C:2 * C]
    wv = w_qkv[:, 2 * C:3 * C]

    in_maps = []
    for c in range(N_CORES):
        b = c // GROUPS
        hg = c % GROUPS
        hs = slice(hg * HPC * HD, (hg + 1) * HPC * HD)
        wq_c = wq[:, hs].reshape(C, HPC, HD)[:, :, perm].reshape(C, HPC * HD)
        wk_c = wk[:, hs].reshape(C, HPC, HD)[:, :, perm].reshape(C, HPC * HD)
        in_maps.append({
            "xT": np.ascontiguousarray(x[b].T).astype(np.float16),
            "wqk": np.concatenate([wq_c, wk_c], axis=1).astype(np.float16),
            "wv": np.ascontiguousarray(wv[:, hs]).astype(np.float16),
            "wp": np.ascontiguousarray(w_proj[hs, :]).astype(np.float16),
            "cs2": cs2,
            "sc2": sc2,
            "masks": masks,
        })
    return in_maps


class _PjrtRunner:
    """Caches the jitted shard_map callable so repeat kernel() calls skip
    retracing. Mirrors concourse.bass2jax.run_bass_via_pjrt."""

    def __init__(self, nc):
        import jax
        from jax.sharding import Mesh, PartitionSpec, NamedSharding
        from jax.experimental.shard_map import shard_map
        from concourse.bass2jax import (
            _bass_exec_p, install_neuronx_cc_hook, partition_id_tensor)

        install_neuronx_cc_hook()
        self.jax = jax
        partition_name = nc.partition_id_tensor.name if nc.partition_id_tensor else None
        in_names, out_names, out_avals = [], [], []
        for alloc in nc.m.functions[0].allocations:
            if not isinstance(alloc, mybir.MemoryLocationSet):
                continue
            name = alloc.memorylocations[0].name
            if alloc.kind == "ExternalInput":
                if name != partition_name:
                    in_names.append(name)
            elif alloc.kind == "ExternalOutput":
                out_names.append(name)
                out_avals.append(jax.core.ShapedArray(
                    tuple(alloc.tensor_shape), mybir.dt.np(alloc.dtype)))
        self.in_names, self.out_names, self.out_avals = in_names, out_names, out_avals
        n_params = len(in_names)
        n_outs = len(out_avals)
        bind_names = tuple(in_names + out_names +
                           ([partition_name] if partition_name else []))
        donate = tuple(range(n_params, n_params + n_outs))

        def _body(*args):
            operands = list(args)
            if partition_name:
                operands.append(partition_id_tensor())
            outs = _bass_exec_p.bind(
                *operands,
                out_avals=tuple(out_avals),
                in_names=bind_names,
                out_names=tuple(out_names),
                lowering_input_output_aliases=(),
                sim_require_finite=True,
                sim_require_nnan=True,
                nc=nc,
            )
            return tuple(outs)

        devices = jax.devices()[:N_CORES]
        self.mesh = Mesh(np.asarray(devices), ("core",))
        self.sharding = NamedSharding(self.mesh, PartitionSpec("core"))
        in_specs = (PartitionSpec("core"),) * (n_params + n_outs)
        out_specs = (PartitionSpec("core"),) * len(out_names)
        self.fn = jax.jit(
            shard_map(_body, mesh=self.mesh, in_specs=in_specs,
                      out_specs=out_specs, check_rep=False),
            donate_argnums=donate,
        )

    def run(self, in_maps):
        jax = self.jax
        concat = [
            np.concatenate([np.asarray(m[name]) for m in in_maps], axis=0)
            for name in self.in_names
        ]
        dev = [jax.device_put(c, self.sharding) for c in concat]
        zeros = [
            jax.device_put(
                np.zeros((N_CORES * a.shape[0], *a.shape[1:]), a.dtype),
                self.sharding)
            for a in self.out_avals
        ]
        outs = self.fn(*dev, *zeros)
        jax.block_until_ready(outs)
        res = []
        for c in range(N_CORES):
            d = {}
            for i, name in enumerate(self.out_names):
                a = np.asarray(outs[i])
                d[name] = a.reshape(N_CORES, *self.out_avals[i].shape)[c]
            res.append(d)
        return res


def _get_rt():
    if "rt" not in _CACHE:
        nc = _build_nc(T=T, C=C, HPC=HPC, n_cores=N_CORES, qn=QN, reps=1, an=AN)
        _CACHE["nc"] = nc
        _CACHE["rt"] = _PjrtRunner(nc) if axon_active() else None
    return _CACHE.get("nc"), _CACHE.get("rt")


def kernel(x, w_qkv, w_proj, n_head):
    assert int(n_head) == NH
    x = np.asarray(x, dtype=np.float32)
    w_qkv = np.asarray(w_qkv, dtype=np.float32)
    w_proj = np.asarray(w_proj, dtype=np.float32)
    assert x.shape == (B, T, C) and w_qkv.shape == (C, 3 * C) and w_proj.shape == (C, C)

    nc, rt = _get_rt()
    in_maps = _host_prep(x, w_qkv, w_proj)
    if rt is not None:
        results = rt.run(in_maps)
    else:
        results = run_bass_kernel_spmd(nc, in_maps,
                                       core_ids=list(range(N_CORES))).results

    out = np.zeros((B, T, C), dtype=np.float32)
    for c in range(N_CORES):
        b = c // GROUPS
        out[b] += results[c]["outT"].astype(np.float32).T
    return out
